# revision 22
# baseline (speedup 1.0000x reference)
import sys

sys.path.insert(0, "/opt/trn_rl_repo")
import numpy as np

import concourse.bacc as bacc
import concourse.mybir as mybir
import concourse.tile as tile
from concourse import bass_utils
from concourse._compat import axon_active

f32 = mybir.dt.float32
f16 = mybir.dt.float16

B, H, W, C = 4, 64, 64, 512
N = H * W          # 4096 rows per batch
NOWN = N // 2      # 2048 rows owned per core
D = 64             # qk head dim
NCORES = 8
EOFF = 90.0        # softmax energy offset: exp(e - EOFF); safe window [46, 135]

TRACE = False
LAST_EXEC_NS = None

_CACHE = {}


def _build(gamma_f, rep=1):
    nc = bacc.Bacc(
        "TRN2", target_bir_lowering=False, debug=not axon_active(), num_devices=1
    )
    # host-packed layouts (see _in_maps):
    #   xT_p[p, cb*N + n]   = x[n, cb*128 + p]          (f32, for q/k proj)
    #   xb_p[p, jc*C + c]   = x[jc*128 + p, c]          (f32, attn @ x)
    #   w_p[p, cb*128 + m]  = [Wq|Wk][cb*128 + p, m]    (f32)
    #   w_p2[p, cb*C + c]   = Wv[cb*128 + p, c]         (f32)
    #   out_p[it*128 + p, s*C + c] = out[it*512 + s*128 + p, c]
    xt_d = nc.dram_tensor("xTp", [128, 4 * N], f32, kind="ExternalInput").ap()
    xb_d = nc.dram_tensor("xbp", [128, 32 * C], f32, kind="ExternalInput").ap()
    wqk_d = nc.dram_tensor("wqkp", [128, 4 * 128], f32, kind="ExternalInput").ap()
    wv_d = nc.dram_tensor("wvp", [128, 4 * C], f32, kind="ExternalInput").ap()
    out_d = nc.dram_tensor("out", [512, 4 * C], f32, kind="ExternalOutput").ap()

    MUL = mybir.AluOpType.mult

    with tile.TileContext(nc) as tc:
        with tc.tile_pool(name="sb", bufs=1) as pool, tc.tile_pool(
            name="ps", bufs=1, space="PSUM"
        ) as psum:
            xT = pool.tile([128, 4 * N], f32)
            x_big = pool.tile([128, 32 * C], f32)
            wqk_sb = pool.tile([128, 4 * 128], f32)
            wv_sb = pool.tile([128, 4 * C], f32)
            qkT = pool.tile([128, N], f32)       # rows 0..63 qT, 64..127 kT
            kT = pool.tile([D, NOWN], f32)       # kT own rows at base partition 0
            uT_sb = pool.tile([128, 4 * 512], f32)
            ones_c = pool.tile([128, 1], f32)
            negoff = pool.tile([128, 1], f32)
            zrec = pool.tile([1, C], f32)
            zrT = pool.tile([128, 4], f32)
            nc.vector.memset(negoff, -EOFF)
            nc.vector.memset(ones_c, 1.0)

            with tc.For_i(0, rep, 1) as _i:
                # ---- load ----
                nc.sync.dma_start(xT, xt_d)
                nc.sync.dma_start(x_big, xb_d)
                nc.sync.dma_start(wqk_sb, wqk_d)
                nc.sync.dma_start(wv_sb, wv_d)

                # ---- qk projection: qkT rows 0..63 = qT, 64..127 = kT ----
                for ch in range(4):  # 1024-col chunks of n
                    pq = psum.tile([128, 1024], f32, tag="eps", bufs=1)
                    for half in range(2):
                        lo = ch * 1024 + half * 512
                        for cb in range(4):
                            nc.tensor.matmul(
                                pq[:, half * 512 : (half + 1) * 512],
                                wqk_sb[:, cb * 128 : (cb + 1) * 128],
                                xT[:, cb * N + lo : cb * N + lo + 512],
                                start=(cb == 0),
                                stop=(cb == 3),
                            )
                    nc.vector.tensor_copy(qkT[:, ch * 1024 : (ch + 1) * 1024], pq)
                nc.sync.dma_start(kT, qkT[D:128, 0:NOWN])

                # ---- attention over own i rows, it-tiles of 512 ----
                for it in range(4):
                    uT = [
                        psum.tile([128, 512], f32, tag="uT", bufs=4, name=f"uT{s}")
                        for s in range(4)
                    ]
                    zrow = psum.tile([1, C], f32, tag="zrow", bufs=1)
                    for jp in range(16):
                        eps = psum.tile([128, 1024], f32, tag="eps", bufs=1)
                        for u in range(2):
                            nc.tensor.matmul(
                                eps[:, u * 512 : (u + 1) * 512],
                                qkT[0:D, (2 * jp + u) * 128 : (2 * jp + u + 1) * 128],
                                kT[:, it * 512 : (it + 1) * 512],
                                start=True,
                                stop=True,
                            )
                        st = pool.tile([128, 1024], f32, tag="st", bufs=2)
                        nc.scalar.activation(
                            st, eps, mybir.ActivationFunctionType.Exp,
                            bias=negoff[:, 0:1],
                        )
                        for u in range(2):
                            jc = 2 * jp + u
                            for cc in range(4):
                                nc.tensor.matmul(
                                    uT[cc],
                                    x_big[:, jc * C + cc * 128 : jc * C + (cc + 1) * 128],
                                    st[:, u * 512 : (u + 1) * 512],
                                    start=(jc == 0),
                                    stop=(jc == 31),
                                )
                            nc.tensor.matmul(
                                zrow,
                                ones_c,
                                st[:, u * 512 : (u + 1) * 512],
                                start=(jp == 0 and u == 0),
                                stop=(jp == 15 and u == 1),
                            )
                    for cc in range(4):
                        nc.vector.tensor_copy(
                            uT_sb[:, cc * 512 : (cc + 1) * 512], uT[cc]
                        )
                    nc.vector.reciprocal(zrec, zrow)
                    for s in range(4):
                        nc.sync.dma_start(
                            zrT[:, s : s + 1], zrec[:, s * 128 : (s + 1) * 128]
                        )
                    ob = pool.tile([128, 4 * C], f32, tag="ob", bufs=2)
                    for s in range(4):
                        fin = psum.tile([128, 1024], f32, tag="eps", bufs=1)
                        for cc in range(4):
                            nc.tensor.matmul(
                                fin[:, 0:512],
                                uT_sb[:, cc * 512 + s * 128 : cc * 512 + (s + 1) * 128],
                                wv_sb[:, cc * C : (cc + 1) * C],
                                start=(cc == 0),
                                stop=(cc == 3),
                            )
                        nc.vector.tensor_scalar(
                            out=ob[:, s * C : (s + 1) * C], in0=fin[:, 0:512],
                            scalar1=zrT[:, s : s + 1], scalar2=gamma_f,
                            op0=MUL, op1=MUL,
                        )
                    nc.sync.dma_start(out_d[it * 128 : (it + 1) * 128, :], ob)

    nc.compile()
    return nc


def _in_maps(x, Wq, Wk, Wv):
    wqk = np.concatenate(
        [np.asarray(Wq), np.asarray(Wk)], axis=1
    ).astype(np.float32)
    # w_p[p, cb*128 + m] = wqk[cb*128 + p, m]
    wqk_p = np.ascontiguousarray(
        wqk.reshape(4, 128, 128).transpose(1, 0, 2).reshape(128, 512)
    )
    wv_p = np.ascontiguousarray(
        np.asarray(Wv, dtype=np.float32)
        .reshape(4, 128, 512).transpose(1, 0, 2).reshape(128, 2048)
    )
    maps = []
    for c in range(NCORES):
        b, h = c // 2, c % 2
        xb = np.asarray(x[b], dtype=np.float32).reshape(N, C)
        xr = np.roll(xb, -h * NOWN, axis=0)
        # xT_p[p, cb*N + n] = xr[n, cb*128 + p]
        xt_p = np.ascontiguousarray(
            xr.T.reshape(4, 128, N).transpose(1, 0, 2).reshape(128, 4 * N)
        )
        # xb_p[p, jc*C + c] = xr[jc*128 + p, c]
        xb_p = np.ascontiguousarray(
            xr.reshape(32, 128, C).transpose(1, 0, 2).reshape(128, 32 * C)
        )
        maps.append({"xTp": xt_p, "xbp": xb_p, "wqkp": wqk_p, "wvp": wv_p})
    return maps


def _gather(results):
    out = np.empty((B, N, C), dtype=np.float32)
    for c in range(NCORES):
        b, h = c // 2, c % 2
        # out_p[it*128 + p, s*C + c] -> rows it*512 + s*128 + p
        arr = results[c]["out"].reshape(4, 128, 4, C).transpose(0, 2, 1, 3)
        out[b, h * NOWN : (h + 1) * NOWN, :] = arr.reshape(NOWN, C)
    return out.reshape(B, H, W, C)


def kernel(x, Wq, Wk, Wv, gamma):
    global LAST_EXEC_NS
    gamma_f = float(np.asarray(gamma).reshape(-1)[0])
    nc = _CACHE.get(gamma_f)
    if nc is None:
        nc = _build(gamma_f)
        _CACHE[gamma_f] = nc

    res = bass_utils.run_bass_kernel_spmd(
        nc, _in_maps(x, Wq, Wk, Wv), core_ids=list(range(NCORES)), trace=TRACE
    )
    LAST_EXEC_NS = getattr(res, "exec_time_ns", None)
    return _gather(res.results)


# revision 23
# speedup vs baseline: 1.4282x; 1.4282x over previous
import sys

sys.path.insert(0, "/opt/trn_rl_repo")
import numpy as np

import concourse.bacc as bacc
import concourse.mybir as mybir
import concourse.tile as tile
from concourse import bass_utils
from concourse._compat import axon_active

f32 = mybir.dt.float32
f16 = mybir.dt.float16

B, H, W, C = 4, 64, 64, 512
N = H * W          # 4096 rows per batch
NOWN = N // 2      # 2048 rows owned per core
D = 64             # qk head dim
NCORES = 8
EOFF = 90.0        # softmax energy offset: exp(e - EOFF); safe window [46, 135]

TRACE = False
LAST_EXEC_NS = None

_CACHE = {}


def _build(gamma_f, rep=1):
    nc = bacc.Bacc(
        "TRN2", target_bir_lowering=False, debug=not axon_active(), num_devices=1
    )
    # host-packed layouts (see _in_maps):
    #   xT_p[p, cb*N + n]   = x[n, cb*128 + p]          (f16, for q/k proj)
    #   xb_p[p, jc*C + c]   = x[jc*128 + p, c]          (f32, attn @ x)
    #   w_p[p, cb*128 + m]  = [Wq|Wk][cb*128 + p, m]    (f16)
    #   w_p2[p, cb*C + c]   = Wv[cb*128 + p, c]         (f32)
    #   out_p[it*128 + p, s*C + c] = out[it*512 + s*128 + p, c]
    xt_d = nc.dram_tensor("xTp", [128, 4 * N], f16, kind="ExternalInput").ap()
    xb_d = nc.dram_tensor("xbp", [128, 32 * C], f32, kind="ExternalInput").ap()
    wqk_d = nc.dram_tensor("wqkp", [128, 4 * 128], f16, kind="ExternalInput").ap()
    wv_d = nc.dram_tensor("wvp", [128, 4 * C], f32, kind="ExternalInput").ap()
    out_d = nc.dram_tensor("out", [512, 4 * C], f32, kind="ExternalOutput").ap()

    MUL = mybir.AluOpType.mult

    with tile.TileContext(nc) as tc:
        with tc.tile_pool(name="sb", bufs=1) as pool, tc.tile_pool(
            name="ps", bufs=1, space="PSUM"
        ) as psum:
            xT = pool.tile([128, 4 * N], f16)
            x_big = pool.tile([128, 32 * C], f32)
            wqk_sb = pool.tile([128, 4 * 128], f16)
            wv_sb = pool.tile([128, 4 * C], f32)
            qkT = pool.tile([128, N], f32)       # rows 0..63 qT, 64..127 kT
            kT = pool.tile([D, NOWN], f32)       # kT own rows at base partition 0
            uT_sb = pool.tile([128, 4 * 512], f32)
            ones_c = pool.tile([128, 1], f32)
            negoff = pool.tile([128, 1], f32)
            zrec = pool.tile([1, C], f32)
            zrT = pool.tile([128, 4], f32)
            nc.vector.memset(negoff, -EOFF)
            nc.vector.memset(ones_c, 1.0)

            with tc.For_i(0, rep, 1) as _i:
                # ---- load ----
                nc.sync.dma_start(xT, xt_d)
                nc.sync.dma_start(x_big, xb_d)
                nc.sync.dma_start(wqk_sb, wqk_d)
                nc.sync.dma_start(wv_sb, wv_d)

                # ---- qk projection: qkT rows 0..63 = qT, 64..127 = kT ----
                for ch in range(4):  # 1024-col chunks of n
                    pq = psum.tile([128, 1024], f32, tag="eps", bufs=1)
                    for half in range(2):
                        lo = ch * 1024 + half * 512
                        for cb in range(4):
                            nc.tensor.matmul(
                                pq[:, half * 512 : (half + 1) * 512],
                                wqk_sb[:, cb * 128 : (cb + 1) * 128],
                                xT[:, cb * N + lo : cb * N + lo + 512],
                                start=(cb == 0),
                                stop=(cb == 3),
                            )
                    nc.vector.tensor_copy(qkT[:, ch * 1024 : (ch + 1) * 1024], pq)
                nc.sync.dma_start(kT, qkT[D:128, 0:NOWN])

                # ---- attention over own i rows, it-tiles of 512 ----
                for it in range(4):
                    uT = [
                        psum.tile([128, 512], f32, tag="uT", bufs=4, name=f"uT{s}")
                        for s in range(4)
                    ]
                    zrow = psum.tile([1, C], f32, tag="zrow", bufs=1)
                    for jp in range(16):
                        eps = psum.tile([128, 1024], f32, tag="eps", bufs=1)
                        for u in range(2):
                            nc.tensor.matmul(
                                eps[:, u * 512 : (u + 1) * 512],
                                qkT[0:D, (2 * jp + u) * 128 : (2 * jp + u + 1) * 128],
                                kT[:, it * 512 : (it + 1) * 512],
                                start=True,
                                stop=True,
                            )
                        st = pool.tile([128, 1024], f32, tag="st", bufs=2)
                        nc.scalar.activation(
                            st, eps, mybir.ActivationFunctionType.Exp,
                            bias=negoff[:, 0:1],
                        )
                        for u in range(2):
                            jc = 2 * jp + u
                            for cc in range(4):
                                nc.tensor.matmul(
                                    uT[cc],
                                    x_big[:, jc * C + cc * 128 : jc * C + (cc + 1) * 128],
                                    st[:, u * 512 : (u + 1) * 512],
                                    start=(jc == 0),
                                    stop=(jc == 31),
                                )
                            nc.tensor.matmul(
                                zrow,
                                ones_c,
                                st[:, u * 512 : (u + 1) * 512],
                                start=(jp == 0 and u == 0),
                                stop=(jp == 15 and u == 1),
                            )
                    for cc in range(4):
                        nc.vector.tensor_copy(
                            uT_sb[:, cc * 512 : (cc + 1) * 512], uT[cc]
                        )
                    nc.vector.reciprocal(zrec, zrow)
                    for s in range(4):
                        nc.sync.dma_start(
                            zrT[:, s : s + 1], zrec[:, s * 128 : (s + 1) * 128]
                        )
                    ob = pool.tile([128, 4 * C], f32, tag="ob", bufs=2)
                    for s in range(4):
                        fin = psum.tile([128, 1024], f32, tag="eps", bufs=1)
                        for cc in range(4):
                            nc.tensor.matmul(
                                fin[:, 0:512],
                                uT_sb[:, cc * 512 + s * 128 : cc * 512 + (s + 1) * 128],
                                wv_sb[:, cc * C : (cc + 1) * C],
                                start=(cc == 0),
                                stop=(cc == 3),
                            )
                        nc.vector.tensor_scalar(
                            out=ob[:, s * C : (s + 1) * C], in0=fin[:, 0:512],
                            scalar1=zrT[:, s : s + 1], scalar2=gamma_f,
                            op0=MUL, op1=MUL,
                        )
                    nc.sync.dma_start(out_d[it * 128 : (it + 1) * 128, :], ob)

    nc.compile()
    return nc


def _in_maps(x, Wq, Wk, Wv):
    wqk = np.concatenate(
        [np.asarray(Wq), np.asarray(Wk)], axis=1
    ).astype(np.float32)
    # w_p[p, cb*128 + m] = wqk[cb*128 + p, m]
    wqk_p = np.ascontiguousarray(
        wqk.reshape(4, 128, 128).transpose(1, 0, 2).reshape(128, 512)
    ).astype(np.float16)
    wv_p = np.ascontiguousarray(
        np.asarray(Wv, dtype=np.float32)
        .reshape(4, 128, 512).transpose(1, 0, 2).reshape(128, 2048)
    )
    maps = []
    for c in range(NCORES):
        b, h = c // 2, c % 2
        xb = np.asarray(x[b], dtype=np.float32).reshape(N, C)
        xr = np.roll(xb, -h * NOWN, axis=0)
        # xT_p[p, cb*N + n] = xr[n, cb*128 + p]
        xt_p = np.ascontiguousarray(
            xr.T.reshape(4, 128, N).transpose(1, 0, 2).reshape(128, 4 * N)
        ).astype(np.float16)
        # xb_p[p, jc*C + c] = xr[jc*128 + p, c]
        xb_p = np.ascontiguousarray(
            xr.reshape(32, 128, C).transpose(1, 0, 2).reshape(128, 32 * C)
        )
        maps.append({"xTp": xt_p, "xbp": xb_p, "wqkp": wqk_p, "wvp": wv_p})
    return maps


def _gather(results):
    out = np.empty((B, N, C), dtype=np.float32)
    for c in range(NCORES):
        b, h = c // 2, c % 2
        # out_p[it*128 + p, s*C + c] -> rows it*512 + s*128 + p
        arr = results[c]["out"].reshape(4, 128, 4, C).transpose(0, 2, 1, 3)
        out[b, h * NOWN : (h + 1) * NOWN, :] = arr.reshape(NOWN, C)
    return out.reshape(B, H, W, C)


def kernel(x, Wq, Wk, Wv, gamma):
    global LAST_EXEC_NS
    gamma_f = float(np.asarray(gamma).reshape(-1)[0])
    nc = _CACHE.get(gamma_f)
    if nc is None:
        nc = _build(gamma_f)
        _CACHE[gamma_f] = nc

    res = bass_utils.run_bass_kernel_spmd(
        nc, _in_maps(x, Wq, Wk, Wv), core_ids=list(range(NCORES)), trace=TRACE
    )
    LAST_EXEC_NS = getattr(res, "exec_time_ns", None)
    return _gather(res.results)
